# revision 34
# baseline (speedup 1.0000x reference)
"""Linformer multi-head self-attention on 8 Trainium2 NeuronCores.

Sharding: data-parallel over batch (4) x tensor-parallel over head groups (2).
Core c handles batch c//2, heads (c%2)*8 .. (c%2)*8+8 (channel block of 512).
Each core computes a partial output [4096, 1024]; the host sums the two
partials per batch.

Per-core algorithm (Linformer algebraic reformulation — K and V are never
materialized; only their low-rank projections are):
  C. qT[j, n]      = wq_slice @ x.T for ALL n (runs FIRST: needs only wq +
                     xT, so the PE starts ~7us after launch while the A-phase
                     inputs stream in behind it)
  A. XtEF[f, e2]   = x.T @ [proj_e | proj_f]            (contract n)
  B. kpT[d, e]     = wk_slice @ XtE   (per head-pair)   (contract f)
     vp[e, d]      = XtF.T @ wv_slice.T                 (contract f)
     vaug          = [1 | 0*63 | vp] per head: E-matmul output row 0 is the
                     softmax denominator, rows 64:128 the unnormalized oT
  D. sT[e, n]      = kpT.T @ qT  per head -> one 2-bank PSUM tile; a single
                     exp(sT/8) activation covers both e-chunks (bounded
                     scores, max-subtraction provably unnecessary here)
  E. ps[0|64:128]  = vaug.T @ expT  (row 0 = denominator, rows 64:128 = oT)
     norm: reciprocal_approx_fast (DVE) -> partition_broadcast (Pool) ->
     tensor_mul (DVE).  No PE broadcast matmul, no full-rate reciprocal.
  F. y[n, g]      += outT.T @ wo_slice.T                (contract j)

D/E/F are software-pipelined over 64 (block, head) slots: slot i emits
mul(i-2), D(i), E(i-1)+recip+bcast, F(i-12) so the tensor engine always has
ready work queued ahead of any instruction that waits on ACT/DVE consumers.
The mul is deferred one slot relative to its recip/bcast so the in-order
DVE queue never parks a ready reciprocal behind a mul that is still waiting
on the Pool broadcast (that head-of-line block was worth ~10us).
PSUM: 2x 2-bank score tiles + 2 attention-out banks + 2 output banks.
Block 0 of C' accumulates in two fc half-passes so the very first matmul
needs only the first half of the wq/xT0 DMAs.
"""

import sys

sys.path.insert(0, "/opt/trn_rl_repo")

import numpy as np
import ml_dtypes

import concourse.bass as bass  # noqa: F401  (AP helpers)
import concourse.mybir as mybir
import concourse.tile as tile
from concourse import bacc
from concourse.bass_utils import run_bass_kernel_spmd

SEQ = 4096
FEAT = 1024
PD = 256          # linformer projection dim
J = 512           # per-core head channels (8 heads x 64)
HD = 64           # head dim
NB = 512          # token block for fused loop
N_BLOCKS = SEQ // NB          # 8
NT_PER_BLOCK = NB // 128      # 4
FC = FEAT // 128              # 8 feature chunks
TAU_INV = 1.0 / 8.0           # 1/sqrt(HD)
F_LAG = 12
HEAD_ORDER = [0, 2, 4, 6, 1, 3, 5, 7]

BF16 = mybir.dt.bfloat16
F32 = mybir.dt.float32
NPBF16 = ml_dtypes.bfloat16


def build_nc():
    nc = bacc.Bacc("TRN2", target_bir_lowering=False, debug=False)

    xT = nc.dram_tensor("xT", [FEAT, SEQ], BF16, kind="ExternalInput")
    xn = nc.dram_tensor("xn", [SEQ, FEAT], BF16, kind="ExternalInput")
    pef = nc.dram_tensor("pef", [SEQ, 2 * PD], BF16, kind="ExternalInput")
    wqT = nc.dram_tensor("wqT", [FEAT, J], BF16, kind="ExternalInput")
    wkT = nc.dram_tensor("wkT", [FEAT, J], BF16, kind="ExternalInput")
    wvT = nc.dram_tensor("wvT", [FEAT, J], BF16, kind="ExternalInput")
    woT = nc.dram_tensor("woT", [J, FEAT], BF16, kind="ExternalInput")
    y = nc.dram_tensor("y", [SEQ, FEAT], F32, kind="ExternalOutput")

    with tile.TileContext(nc) as tc:
        _body(nc, tc, xT, xn, pef, wqT, wkT, wvT, woT, y)
    nc.compile()
    return nc


def _body(nc, tc, xT, xn, pef, wqT, wkT, wvT, woT, y):
    Exp = mybir.ActivationFunctionType.Exp

    with (
        tc.tile_pool(name="consts", bufs=1) as consts,
        tc.tile_pool(name="xn_pool", bufs=4) as xn_pool,
        tc.tile_pool(name="pef_pool", bufs=4) as pef_pool,
        tc.tile_pool(name="xtnb", bufs=4) as xtnb_pool,
        tc.tile_pool(name="expp", bufs=4) as exp_pool,
        tc.tile_pool(name="recp", bufs=3) as rec_pool,
        tc.tile_pool(name="bcp", bufs=3) as bc_pool,
        tc.tile_pool(name="outt", bufs=3) as outt_pool,
        tc.tile_pool(name="yp", bufs=3) as y_pool,
    ):
        # ---- resident constants -------------------------------------------
        wq_sb = consts.tile([128, FC, J], BF16, tag="wq")
        wk_sb = consts.tile([128, FC, J], BF16, tag="wk")
        wv_sb = consts.tile([128, FC, J], BF16, tag="wv")
        wo_sb = consts.tile([128, 4, FEAT], BF16, tag="wo")

        xtef_sb = consts.tile([128, FC, 2 * PD], BF16, tag="xtef")
        kpt_sb = consts.tile([128, 4, PD], BF16, tag="kpt")
        # E-matmul lhsT per (ec, head): col 0 = ones (denominator row),
        # cols 1:64 = zeros, cols 64:128 = vp (attention-out rows 64:128)
        vaug_sb = consts.tile([128, 2, 8, 128], BF16, tag="vaug")
        nc.vector.memset(vaug_sb[:], 0.0)
        nc.vector.memset(vaug_sb[:, :, :, 0:1], 1.0)
        qt_all = consts.tile([128, 4, SEQ], BF16, tag="qt_all")

        xn3 = xn[:].rearrange("(t p) f -> p t f", p=128)    # [128, 32, 1024]
        pef3 = pef[:].rearrange("(t p) e -> p t e", p=128)  # [128, 32, 512]
        y3 = y[:].rearrange("(t p) g -> p t g", p=128)      # [128, 32, 1024]
        xT3 = xT[:].rearrange("(c p) n -> p c n", p=128)    # [128, 8, 4096]

        # C' needs only wq chunk 0 + the first xT slice to start: split both
        # loads so the first matmul isn't gated on the full 2 MB
        wqT3 = wqT[:].rearrange("(c p) j -> p c j", p=128)

        # ---- phase C': qT for ALL tokens (PE warm-up phase) ---------------
        # A-phase inputs (12 MB of xn/pef) and remaining weights stream in
        # behind C''s compute.
        xn_tiles = []
        pef_tiles = []
        xt_tiles = []

        def _load_xt(nb):
            xt_nb = xtnb_pool.tile([128, FC, NB], BF16, tag="xtnb", name=f"xt{nb}")
            nc.sync.dma_start(out=xt_nb[:], in_=xT3[:, :, nb * NB : (nb + 1) * NB])
            xt_tiles.append(xt_nb)

        def _load_xnpef(q):
            xn_t = xn_pool.tile([128, 4, FEAT], BF16, tag="xn", name=f"xn{q}")
            pef_t = pef_pool.tile([128, 4, 2 * PD], BF16, tag="pef", name=f"pef{q}")
            nc.sync.dma_start(out=xn_t[:], in_=xn3[:, q * 4 : (q + 1) * 4, :])
            nc.sync.dma_start(out=pef_t[:], in_=pef3[:, q * 4 : (q + 1) * 4, :])
            xn_tiles.append(xn_t)
            pef_tiles.append(pef_t)

        xt0 = xtnb_pool.tile([128, FC, NB], BF16, tag="xtnb", name="xt0")
        for half in range(2):
            nc.sync.dma_start(
                out=wq_sb[:, 4 * half : 4 * half + 4, :],
                in_=wqT3[:, 4 * half : 4 * half + 4, :],
            )
            nc.sync.dma_start(
                out=xt0[:, 4 * half : 4 * half + 4, :],
                in_=xT3[:, 4 * half : 4 * half + 4, 0:NB],
            )
        xt_tiles.append(xt0)
        for nb in range(1, 4):
            _load_xt(nb)
        with tc.tile_pool(name="psC", bufs=1, space="PSUM") as psC_pool:
            for nb in range(N_BLOCKS):
                xt_nb = xt_tiles[nb]
                # stream later xT blocks and the A/B/F-phase inputs behind
                # this block's compute, in need-order
                if nb < 4:
                    _load_xt(nb + 4)
                if nb == 0:
                    _load_xnpef(0)
                    _load_xnpef(1)
                elif nb == 1:
                    _load_xnpef(2)
                    _load_xnpef(3)
                elif nb == 2:
                    nc.sync.dma_start(
                        out=wk_sb[:], in_=wkT[:].rearrange("(c p) j -> p c j", p=128)
                    )
                    nc.sync.dma_start(
                        out=wv_sb[:], in_=wvT[:].rearrange("(c p) j -> p c j", p=128)
                    )
                elif nb == 3:
                    nc.sync.dma_start(
                        out=wo_sb[:], in_=woT[:].rearrange("(c p) g -> p c g", p=128)
                    )
                qtiles = [
                    psC_pool.tile(
                        [128, NB], F32, tag=f"q{(nb % 2) * 4 + jc}", name=f"q{nb}_{jc}"
                    )
                    for jc in range(4)
                ]
                # block 0: two half-passes over fc so the first matmuls need
                # only the first halves of the wq/xT0 loads
                fc_passes = [range(0, 4), range(4, 8)] if nb == 0 else [range(FC)]
                for fcs in fc_passes:
                    for jc in range(4):
                        for fc in fcs:
                            nc.tensor.matmul(
                                qtiles[jc][:],
                                lhsT=wq_sb[:, fc, jc * 128 : (jc + 1) * 128],
                                rhs=xt_nb[:, fc, :],
                                start=(fc == 0),
                                stop=(fc == FC - 1),
                            )
                for jc in range(4):
                    dst = qt_all[:, jc, nb * NB : (nb + 1) * NB]
                    if jc % 2 == 0:
                        nc.vector.tensor_copy(out=dst, in_=qtiles[jc][:])
                    else:
                        nc.scalar.copy(out=dst, in_=qtiles[jc][:])

        # ---- phase A: XtEF = x.T @ [pe|pf] --------------------------------
        with tc.tile_pool(name="psA", bufs=1, space="PSUM") as psA_pool:
            psA = [
                psA_pool.tile([128, 2 * PD], F32, tag=f"ef{i}", name=f"ef{i}")
                for i in range(FC)
            ]
            for q in range(8):
                if q < 4:
                    xn_t, pef_t = xn_tiles[q], pef_tiles[q]
                else:
                    xn_t = xn_pool.tile([128, 4, FEAT], BF16, tag="xn", name=f"xn{q}")
                    pef_t = pef_pool.tile([128, 4, 2 * PD], BF16, tag="pef", name=f"pef{q}")
                    nc.sync.dma_start(out=xn_t[:], in_=xn3[:, q * 4 : (q + 1) * 4, :])
                    nc.sync.dma_start(out=pef_t[:], in_=pef3[:, q * 4 : (q + 1) * 4, :])
                for t in range(4):
                    nci = q * 4 + t
                    for fc in range(FC):
                        nc.tensor.matmul(
                            psA[fc][:],
                            lhsT=xn_t[:, t, fc * 128 : (fc + 1) * 128],
                            rhs=pef_t[:, t, :],
                            start=(nci == 0),
                            stop=(nci == 31),
                        )
            for fc in range(FC):
                if fc % 2 == 0:
                    nc.vector.tensor_copy(out=xtef_sb[:, fc, :], in_=psA[fc][:])
                else:
                    nc.scalar.copy(out=xtef_sb[:, fc, :], in_=psA[fc][:])

        # ---- phase B: kpT (per head pair) and vaug ------------------------
        with tc.tile_pool(name="psB", bufs=2, space="PSUM") as psB_pool:
            def _emit_kp(p):
                ps = psB_pool.tile([128, PD], F32, tag="kp", name=f"kp{p}")
                for fc in range(FC):
                    nc.tensor.matmul(
                        ps[:],
                        lhsT=wk_sb[:, fc, p * 128 : (p + 1) * 128],
                        rhs=xtef_sb[:, fc, 0:PD],
                        start=(fc == 0),
                        stop=(fc == FC - 1),
                    )
                if p % 2 == 0:
                    nc.vector.tensor_copy(out=kpt_sb[:, p, :], in_=ps[:])
                else:
                    nc.scalar.copy(out=kpt_sb[:, p, :], in_=ps[:])

            def _emit_vp(ec):
                ps = psB_pool.tile([128, J], F32, tag="vp", name=f"vp{ec}")
                for fc in range(FC):
                    nc.tensor.matmul(
                        ps[:],
                        lhsT=xtef_sb[:, fc, PD + ec * 128 : PD + (ec + 1) * 128],
                        rhs=wv_sb[:, fc, :],
                        start=(fc == 0),
                        stop=(fc == FC - 1),
                    )
                for h in range(8):
                    if h % 2 == 0:
                        nc.vector.tensor_copy(
                            out=vaug_sb[:, ec, h, 64:128],
                            in_=ps[:, h * HD : (h + 1) * HD],
                        )
                    else:
                        nc.scalar.copy(
                            out=vaug_sb[:, ec, h, 64:128],
                            in_=ps[:, h * HD : (h + 1) * HD],
                        )

            _emit_kp(0)
            _emit_vp(0)
            _emit_kp(1)
            _emit_vp(1)
            _emit_kp(2)
            _emit_kp(3)

        # ---- fused D/E/F per token block of 512, software-pipelined -------
        N_SLOTS = N_BLOCKS * 8
        with tc.tile_pool(name="psM", bufs=1, space="PSUM") as psM_pool:
            outt_tiles = {}
            ysb_cur = [None]

            def emit_D(i):
                nb, h = i // 8, HEAD_ORDER[i % 8]
                p, off = h // 2, (h % 2) * 64
                ex = exp_pool.tile([128, 2, NB], BF16, tag="exp", name=f"ex{i}")
                ps_s = psM_pool.tile(
                    [128, 2, NB], F32, tag=f"s{i % 2}", name=f"ps_s{i}"
                )
                for ec in range(2):
                    nc.tensor.matmul(
                        ps_s[:, ec, :],
                        lhsT=kpt_sb[off : off + 64, p, ec * 128 : (ec + 1) * 128],
                        rhs=qt_all[off : off + 64, p, nb * NB : (nb + 1) * NB],
                        start=True,
                        stop=True,
                    )
                nc.scalar.activation(
                    out=ex[:], in_=ps_s[:], func=Exp, scale=TAU_INV
                )
                return ex

            ps_o_tiles = {}
            bc_tiles = {}

            def emit_E_rb(i, ex):
                nb, h = i // 8, HEAD_ORDER[i % 8]
                if i % 8 == 0:
                    outt_tiles[nb] = outt_pool.tile(
                        [128, 4, NB], BF16, tag="outt", name=f"outt{nb}"
                    )
                ps_o = psM_pool.tile([128, NB], F32, tag=f"o{i % 2}", name=f"ps_o{i}")
                ps_o_tiles[i] = ps_o
                for ec in range(2):
                    nc.tensor.matmul(
                        ps_o[:],
                        lhsT=vaug_sb[:, ec, h, :],
                        rhs=ex[:, ec, :],
                        start=(ec == 0),
                        stop=(ec == 1),
                    )
                rec = rec_pool.tile([1, NB], F32, tag="rec", name=f"rec{i}")
                bc_sb = bc_pool.tile([HD, NB], F32, tag="bc", name=f"bc{i}")
                bc_tiles[i] = bc_sb
                nc.vector.reciprocal_approx_fast(out=rec[:], in_=ps_o[0:1, :])
                nc.gpsimd.partition_broadcast(bc_sb[:], rec[:])

            def emit_mul(i):
                nb, h = i // 8, HEAD_ORDER[i % 8]
                p, off = h // 2, (h % 2) * 64
                nc.vector.tensor_mul(
                    out=outt_tiles[nb][off : off + 64, p, :],
                    in0=ps_o_tiles.pop(i)[64:128, :],
                    in1=bc_tiles.pop(i),
                )

            def emit_F_group(i):
                nb, g = i // 8, i % 8
                tl, gh = g // 2, g % 2
                outt_nb = outt_tiles[nb]
                if gh == 0:
                    ysb_cur[0] = y_pool.tile([128, FEAT], F32, tag="y", name=f"ysb{i}")
                ysb = ysb_cur[0]
                ps_f = psM_pool.tile([128, NB], F32, tag=f"f{g % 2}", name=f"ps_f{i}")
                for p in range(4):
                    nc.tensor.matmul(
                        ps_f[:],
                        lhsT=outt_nb[:, p, tl * 128 : (tl + 1) * 128],
                        rhs=wo_sb[:, p, gh * NB : (gh + 1) * NB],
                        start=(p == 0),
                        stop=(p == 3),
                    )
                nc.scalar.copy(out=ysb[:, gh * NB : (gh + 1) * NB], in_=ps_f[:])
                if gh == 1:
                    nc.sync.dma_start(
                        out=y3[:, nb * NT_PER_BLOCK + tl, :], in_=ysb[:]
                    )

            ex_prev = None
            for i in range(N_SLOTS + F_LAG):
                if i < N_SLOTS:
                    ex_cur = emit_D(i)
                if i >= 2 and i - 2 < N_SLOTS:
                    emit_mul(i - 2)
                if i >= 1 and i - 1 < N_SLOTS:
                    emit_E_rb(i - 1, ex_prev)
                if i >= F_LAG:
                    emit_F_group(i - F_LAG)
                if i < N_SLOTS:
                    ex_prev = ex_cur


_NC_CACHE = {}


def _get_nc():
    if "nc" not in _NC_CACHE:
        _NC_CACHE["nc"] = build_nc()
    return _NC_CACHE["nc"]


def _in_maps(x, w_q, w_k, w_v, w_o, proj_e, proj_f):
    pef = np.concatenate([proj_e, proj_f], axis=1).astype(NPBF16)
    maps = []
    for c in range(8):
        b, hg = c // 2, c % 2
        xb = np.asarray(x[b], dtype=np.float32)
        sl = slice(hg * J, (hg + 1) * J)
        maps.append(
            {
                "xT": xb.T.astype(NPBF16),
                "xn": xb.astype(NPBF16),
                "pef": pef,
                "wqT": w_q[sl, :].T.astype(NPBF16),
                "wkT": w_k[sl, :].T.astype(NPBF16),
                "wvT": w_v[sl, :].T.astype(NPBF16),
                "woT": w_o[:, sl].T.astype(NPBF16),
            }
        )
    return maps


def kernel(**inputs):
    x = np.asarray(inputs["x"], dtype=np.float32)
    res = run_bass_kernel_spmd(
        _get_nc(),
        _in_maps(
            x,
            np.asarray(inputs["w_q"], dtype=np.float32),
            np.asarray(inputs["w_k"], dtype=np.float32),
            np.asarray(inputs["w_v"], dtype=np.float32),
            np.asarray(inputs["w_o"], dtype=np.float32),
            np.asarray(inputs["proj_e"], dtype=np.float32),
            np.asarray(inputs["proj_f"], dtype=np.float32),
        ),
        core_ids=list(range(8)),
    )
    y = np.empty((4, SEQ, FEAT), np.float32)
    for b in range(4):
        y[b] = res.results[2 * b]["y"] + res.results[2 * b + 1]["y"]
    return y


# revision 36
# speedup vs baseline: 1.0317x; 1.0317x over previous
"""Linformer multi-head self-attention on 8 Trainium2 NeuronCores.

Sharding: data-parallel over batch (4) x tensor-parallel over head groups (2).
Core c handles batch c//2, heads (c%2)*8 .. (c%2)*8+8 (channel block of 512).
Each core computes a partial output [4096, 1024]; the host sums the two
partials per batch.

Per-core algorithm (Linformer algebraic reformulation — K and V are never
materialized; only their low-rank projections are):
  C. qT[j, n]      = wq_slice @ x.T for ALL n (runs FIRST: needs only wq +
                     xT, so the PE starts ~7us after launch while the A-phase
                     inputs stream in behind it)
  A. XtEF[f, e2]   = x.T @ [proj_e | proj_f]            (contract n)
  B. kpT[d, e]     = wk_slice @ XtE   (per head-pair)   (contract f)
     vp[e, d]      = XtF.T @ wv_slice.T                 (contract f)
     vaug          = [1 | 0*63 | vp] per head: E-matmul output row 0 is the
                     softmax denominator, rows 64:128 the unnormalized oT
  D. sT[e, n]      = kpT.T @ qT  per head -> one 2-bank PSUM tile; a single
                     exp(sT/8) activation covers both e-chunks (bounded
                     scores, max-subtraction provably unnecessary here)
  E. ps[0|64:128]  = vaug.T @ expT  (row 0 = denominator, rows 64:128 = oT)
     norm: reciprocal_approx_fast (DVE) -> partition_broadcast (Pool) ->
     tensor_mul (DVE).  No PE broadcast matmul, no full-rate reciprocal.
  F. y[n, g]      += outT.T @ wo_slice.T                (contract j)

D/E/F are software-pipelined over 64 (block, head) slots: slot i emits
mul(i-2), D(i), E(i-1)+recip+bcast, F(i-12) so the tensor engine always has
ready work queued ahead of any instruction that waits on ACT/DVE consumers.
The mul is deferred one slot relative to its recip/bcast so the in-order
DVE queue never parks a ready reciprocal behind a mul that is still waiting
on the Pool broadcast (that head-of-line block was worth ~10us).
PSUM: 2x 2-bank score tiles + 2 attention-out banks + 2 output banks.
Block 0 of C' accumulates in two fc half-passes so the very first matmul
needs only the first half of the wq/xT0 DMAs.
"""

import sys

sys.path.insert(0, "/opt/trn_rl_repo")

import numpy as np
import ml_dtypes

import concourse.bass as bass  # noqa: F401  (AP helpers)
import concourse.mybir as mybir
import concourse.tile as tile
from concourse import bacc
from concourse.bass_utils import run_bass_kernel_spmd

SEQ = 4096
FEAT = 1024
PD = 256          # linformer projection dim
J = 512           # per-core head channels (8 heads x 64)
HD = 64           # head dim
NB = 512          # token block for fused loop
N_BLOCKS = SEQ // NB          # 8
NT_PER_BLOCK = NB // 128      # 4
FC = FEAT // 128              # 8 feature chunks
TAU_INV = 1.0 / 8.0           # 1/sqrt(HD)
F_LAG = 12
HEAD_ORDER = [0, 2, 4, 6, 1, 3, 5, 7]

BF16 = mybir.dt.bfloat16
F32 = mybir.dt.float32
NPBF16 = ml_dtypes.bfloat16


def build_nc():
    nc = bacc.Bacc("TRN2", target_bir_lowering=False, debug=False)

    xT = nc.dram_tensor("xT", [FEAT, SEQ], BF16, kind="ExternalInput")
    xn = nc.dram_tensor("xn", [SEQ, FEAT], BF16, kind="ExternalInput")
    pef = nc.dram_tensor("pef", [SEQ, 2 * PD], BF16, kind="ExternalInput")
    wqT = nc.dram_tensor("wqT", [FEAT, J], BF16, kind="ExternalInput")
    wkT = nc.dram_tensor("wkT", [FEAT, J], BF16, kind="ExternalInput")
    wvT = nc.dram_tensor("wvT", [FEAT, J], BF16, kind="ExternalInput")
    woT = nc.dram_tensor("woT", [J, FEAT], BF16, kind="ExternalInput")
    y = nc.dram_tensor("y", [SEQ, FEAT], F32, kind="ExternalOutput")

    with tile.TileContext(nc) as tc:
        _body(nc, tc, xT, xn, pef, wqT, wkT, wvT, woT, y)
    nc.compile()
    return nc


def _body(nc, tc, xT, xn, pef, wqT, wkT, wvT, woT, y):
    Exp = mybir.ActivationFunctionType.Exp

    with (
        tc.tile_pool(name="consts", bufs=1) as consts,
        tc.tile_pool(name="xn_pool", bufs=4) as xn_pool,
        tc.tile_pool(name="pef_pool", bufs=4) as pef_pool,
        tc.tile_pool(name="xtnb", bufs=5) as xtnb_pool,
        tc.tile_pool(name="expp", bufs=3) as exp_pool,
        tc.tile_pool(name="recp", bufs=2) as rec_pool,
        tc.tile_pool(name="bcp", bufs=3) as bc_pool,
        tc.tile_pool(name="outt", bufs=3) as outt_pool,
        tc.tile_pool(name="yp", bufs=3) as y_pool,
    ):
        # ---- resident constants -------------------------------------------
        wq_sb = consts.tile([128, FC, J], BF16, tag="wq")
        wk_sb = consts.tile([128, FC, J], BF16, tag="wk")
        wv_sb = consts.tile([128, FC, J], BF16, tag="wv")
        wo_sb = consts.tile([128, 4, FEAT], BF16, tag="wo")

        xtef_sb = consts.tile([128, FC, 2 * PD], BF16, tag="xtef")
        kpt_sb = consts.tile([128, 4, PD], BF16, tag="kpt")
        # E-matmul lhsT per (ec, head): col 0 = ones (denominator row),
        # cols 1:64 = zeros, cols 64:128 = vp (attention-out rows 64:128)
        vaug_sb = consts.tile([128, 2, 8, 128], BF16, tag="vaug")
        nc.vector.memset(vaug_sb[:], 0.0)
        nc.vector.memset(vaug_sb[:, :, :, 0:1], 1.0)
        qt_all = consts.tile([128, 4, SEQ], BF16, tag="qt_all")

        xn3 = xn[:].rearrange("(t p) f -> p t f", p=128)    # [128, 32, 1024]
        pef3 = pef[:].rearrange("(t p) e -> p t e", p=128)  # [128, 32, 512]
        y3 = y[:].rearrange("(t p) g -> p t g", p=128)      # [128, 32, 1024]
        xT3 = xT[:].rearrange("(c p) n -> p c n", p=128)    # [128, 8, 4096]

        # C' needs only wq chunk 0 + the first xT slice to start: split both
        # loads so the first matmul isn't gated on the full 2 MB
        wqT3 = wqT[:].rearrange("(c p) j -> p c j", p=128)

        # ---- phase C': qT for ALL tokens (PE warm-up phase) ---------------
        # A-phase inputs (12 MB of xn/pef) and remaining weights stream in
        # behind C''s compute.
        xn_tiles = []
        pef_tiles = []
        xt_tiles = []

        def _load_xt(nb):
            xt_nb = xtnb_pool.tile([128, FC, NB], BF16, tag="xtnb", name=f"xt{nb}")
            nc.sync.dma_start(out=xt_nb[:], in_=xT3[:, :, nb * NB : (nb + 1) * NB])
            xt_tiles.append(xt_nb)

        def _load_xnpef(q):
            xn_t = xn_pool.tile([128, 4, FEAT], BF16, tag="xn", name=f"xn{q}")
            pef_t = pef_pool.tile([128, 4, 2 * PD], BF16, tag="pef", name=f"pef{q}")
            nc.sync.dma_start(out=xn_t[:], in_=xn3[:, q * 4 : (q + 1) * 4, :])
            nc.sync.dma_start(out=pef_t[:], in_=pef3[:, q * 4 : (q + 1) * 4, :])
            xn_tiles.append(xn_t)
            pef_tiles.append(pef_t)

        xt0 = xtnb_pool.tile([128, FC, NB], BF16, tag="xtnb", name="xt0")
        for half in range(2):
            nc.sync.dma_start(
                out=wq_sb[:, 4 * half : 4 * half + 4, :],
                in_=wqT3[:, 4 * half : 4 * half + 4, :],
            )
            nc.sync.dma_start(
                out=xt0[:, 4 * half : 4 * half + 4, :],
                in_=xT3[:, 4 * half : 4 * half + 4, 0:NB],
            )
        xt_tiles.append(xt0)
        for nb in range(1, 4):
            _load_xt(nb)
        with tc.tile_pool(name="psC", bufs=1, space="PSUM") as psC_pool:
            for nb in range(N_BLOCKS):
                xt_nb = xt_tiles[nb]
                # stream later xT blocks and the A/B/F-phase inputs behind
                # this block's compute, in need-order
                if nb < 4:
                    _load_xt(nb + 4)
                if nb == 0:
                    _load_xnpef(0)
                    _load_xnpef(1)
                elif nb == 1:
                    _load_xnpef(2)
                    _load_xnpef(3)
                elif nb == 2:
                    nc.sync.dma_start(
                        out=wk_sb[:], in_=wkT[:].rearrange("(c p) j -> p c j", p=128)
                    )
                    nc.sync.dma_start(
                        out=wv_sb[:], in_=wvT[:].rearrange("(c p) j -> p c j", p=128)
                    )
                elif nb == 3:
                    nc.sync.dma_start(
                        out=wo_sb[:], in_=woT[:].rearrange("(c p) g -> p c g", p=128)
                    )
                qtiles = [
                    psC_pool.tile(
                        [128, NB], F32, tag=f"q{(nb % 2) * 4 + jc}", name=f"q{nb}_{jc}"
                    )
                    for jc in range(4)
                ]
                # block 0: two half-passes over fc so the first matmuls need
                # only the first halves of the wq/xT0 loads
                fc_passes = [range(0, 4), range(4, 8)] if nb == 0 else [range(FC)]
                for fcs in fc_passes:
                    for jc in range(4):
                        for fc in fcs:
                            nc.tensor.matmul(
                                qtiles[jc][:],
                                lhsT=wq_sb[:, fc, jc * 128 : (jc + 1) * 128],
                                rhs=xt_nb[:, fc, :],
                                start=(fc == 0),
                                stop=(fc == FC - 1),
                            )
                for jc in range(4):
                    dst = qt_all[:, jc, nb * NB : (nb + 1) * NB]
                    if jc % 2 == 0:
                        nc.vector.tensor_copy(out=dst, in_=qtiles[jc][:])
                    else:
                        nc.scalar.copy(out=dst, in_=qtiles[jc][:])

        # ---- phase A: XtEF = x.T @ [pe|pf] --------------------------------
        with tc.tile_pool(name="psA", bufs=1, space="PSUM") as psA_pool:
            psA = [
                psA_pool.tile([128, 2 * PD], F32, tag=f"ef{i}", name=f"ef{i}")
                for i in range(FC)
            ]
            for q in range(8):
                if q < 4:
                    xn_t, pef_t = xn_tiles[q], pef_tiles[q]
                else:
                    xn_t = xn_pool.tile([128, 4, FEAT], BF16, tag="xn", name=f"xn{q}")
                    pef_t = pef_pool.tile([128, 4, 2 * PD], BF16, tag="pef", name=f"pef{q}")
                    nc.sync.dma_start(out=xn_t[:], in_=xn3[:, q * 4 : (q + 1) * 4, :])
                    nc.sync.dma_start(out=pef_t[:], in_=pef3[:, q * 4 : (q + 1) * 4, :])
                for t in range(4):
                    nci = q * 4 + t
                    for fc in range(FC):
                        nc.tensor.matmul(
                            psA[fc][:],
                            lhsT=xn_t[:, t, fc * 128 : (fc + 1) * 128],
                            rhs=pef_t[:, t, :],
                            start=(nci == 0),
                            stop=(nci == 31),
                        )
            for fc in range(FC):
                if fc % 2 == 0:
                    nc.vector.tensor_copy(out=xtef_sb[:, fc, :], in_=psA[fc][:])
                else:
                    nc.scalar.copy(out=xtef_sb[:, fc, :], in_=psA[fc][:])

        # ---- phase B: kpT (per head pair) and vaug ------------------------
        with tc.tile_pool(name="psB", bufs=2, space="PSUM") as psB_pool:
            def _emit_kp(p):
                ps = psB_pool.tile([128, PD], F32, tag="kp", name=f"kp{p}")
                for fc in range(FC):
                    nc.tensor.matmul(
                        ps[:],
                        lhsT=wk_sb[:, fc, p * 128 : (p + 1) * 128],
                        rhs=xtef_sb[:, fc, 0:PD],
                        start=(fc == 0),
                        stop=(fc == FC - 1),
                    )
                if p % 2 == 0:
                    nc.vector.tensor_copy(out=kpt_sb[:, p, :], in_=ps[:])
                else:
                    nc.scalar.copy(out=kpt_sb[:, p, :], in_=ps[:])

            def _emit_vp(ec):
                ps = psB_pool.tile([128, J], F32, tag="vp", name=f"vp{ec}")
                for fc in range(FC):
                    nc.tensor.matmul(
                        ps[:],
                        lhsT=xtef_sb[:, fc, PD + ec * 128 : PD + (ec + 1) * 128],
                        rhs=wv_sb[:, fc, :],
                        start=(fc == 0),
                        stop=(fc == FC - 1),
                    )
                for h in range(8):
                    if h % 2 == 0:
                        nc.vector.tensor_copy(
                            out=vaug_sb[:, ec, h, 64:128],
                            in_=ps[:, h * HD : (h + 1) * HD],
                        )
                    else:
                        nc.scalar.copy(
                            out=vaug_sb[:, ec, h, 64:128],
                            in_=ps[:, h * HD : (h + 1) * HD],
                        )

            _emit_kp(0)
            _emit_vp(0)
            _emit_kp(1)
            _emit_vp(1)
            _emit_kp(2)
            _emit_kp(3)

        # ---- fused D/E/F per token block of 512, software-pipelined -------
        N_SLOTS = N_BLOCKS * 8
        with tc.tile_pool(name="psM", bufs=1, space="PSUM") as psM_pool:
            outt_tiles = {}
            ysb_cur = [None]

            def emit_D(i):
                nb, h = i // 8, HEAD_ORDER[i % 8]
                p, off = h // 2, (h % 2) * 64
                ex = exp_pool.tile([128, 2, NB], BF16, tag="exp", name=f"ex{i}")
                ps_s = psM_pool.tile(
                    [128, 2, NB], F32, tag=f"s{i % 2}", name=f"ps_s{i}"
                )
                for ec in range(2):
                    nc.tensor.matmul(
                        ps_s[:, ec, :],
                        lhsT=kpt_sb[off : off + 64, p, ec * 128 : (ec + 1) * 128],
                        rhs=qt_all[off : off + 64, p, nb * NB : (nb + 1) * NB],
                        start=True,
                        stop=True,
                    )
                nc.scalar.activation(
                    out=ex[:], in_=ps_s[:], func=Exp, scale=TAU_INV
                )
                return ex

            ps_o_tiles = {}
            bc_tiles = {}

            def emit_E_rb(i, ex):
                nb, h = i // 8, HEAD_ORDER[i % 8]
                if i % 8 == 0:
                    outt_tiles[nb] = outt_pool.tile(
                        [128, 4, NB], BF16, tag="outt", name=f"outt{nb}"
                    )
                ps_o = psM_pool.tile([128, NB], F32, tag=f"o{i % 2}", name=f"ps_o{i}")
                ps_o_tiles[i] = ps_o
                for ec in range(2):
                    nc.tensor.matmul(
                        ps_o[:],
                        lhsT=vaug_sb[:, ec, h, :],
                        rhs=ex[:, ec, :],
                        start=(ec == 0),
                        stop=(ec == 1),
                    )
                rec = rec_pool.tile([1, NB], F32, tag="rec", name=f"rec{i}")
                bc_sb = bc_pool.tile([HD, NB], F32, tag="bc", name=f"bc{i}")
                bc_tiles[i] = bc_sb
                nc.vector.reciprocal_approx_fast(out=rec[:], in_=ps_o[0:1, :])
                nc.gpsimd.partition_broadcast(bc_sb[:], rec[:])

            def emit_mul(i):
                nb, h = i // 8, HEAD_ORDER[i % 8]
                p, off = h // 2, (h % 2) * 64
                nc.vector.tensor_mul(
                    out=outt_tiles[nb][off : off + 64, p, :],
                    in0=ps_o_tiles.pop(i)[64:128, :],
                    in1=bc_tiles.pop(i),
                )

            def emit_F_group(i):
                nb, g = i // 8, i % 8
                tl, gh = g // 2, g % 2
                outt_nb = outt_tiles[nb]
                if gh == 0:
                    ysb_cur[0] = y_pool.tile([128, FEAT], F32, tag="y", name=f"ysb{i}")
                ysb = ysb_cur[0]
                ps_f = psM_pool.tile([128, NB], F32, tag=f"f{g % 2}", name=f"ps_f{i}")
                for p in range(4):
                    nc.tensor.matmul(
                        ps_f[:],
                        lhsT=outt_nb[:, p, tl * 128 : (tl + 1) * 128],
                        rhs=wo_sb[:, p, gh * NB : (gh + 1) * NB],
                        start=(p == 0),
                        stop=(p == 3),
                    )
                nc.scalar.copy(out=ysb[:, gh * NB : (gh + 1) * NB], in_=ps_f[:])
                if gh == 1:
                    nc.sync.dma_start(
                        out=y3[:, nb * NT_PER_BLOCK + tl, :], in_=ysb[:]
                    )

            ex_prev = None
            for i in range(N_SLOTS + F_LAG):
                if i < N_SLOTS:
                    ex_cur = emit_D(i)
                if i >= 2 and i - 2 < N_SLOTS:
                    emit_mul(i - 2)
                if i >= 1 and i - 1 < N_SLOTS:
                    emit_E_rb(i - 1, ex_prev)
                if i >= F_LAG:
                    emit_F_group(i - F_LAG)
                if i < N_SLOTS:
                    ex_prev = ex_cur


_NC_CACHE = {}


def _get_nc():
    if "nc" not in _NC_CACHE:
        _NC_CACHE["nc"] = build_nc()
    return _NC_CACHE["nc"]


def _in_maps(x, w_q, w_k, w_v, w_o, proj_e, proj_f):
    pef = np.concatenate([proj_e, proj_f], axis=1).astype(NPBF16)
    maps = []
    for c in range(8):
        b, hg = c // 2, c % 2
        xb = np.asarray(x[b], dtype=np.float32)
        sl = slice(hg * J, (hg + 1) * J)
        maps.append(
            {
                "xT": xb.T.astype(NPBF16),
                "xn": xb.astype(NPBF16),
                "pef": pef,
                "wqT": w_q[sl, :].T.astype(NPBF16),
                "wkT": w_k[sl, :].T.astype(NPBF16),
                "wvT": w_v[sl, :].T.astype(NPBF16),
                "woT": w_o[:, sl].T.astype(NPBF16),
            }
        )
    return maps


def kernel(**inputs):
    x = np.asarray(inputs["x"], dtype=np.float32)
    res = run_bass_kernel_spmd(
        _get_nc(),
        _in_maps(
            x,
            np.asarray(inputs["w_q"], dtype=np.float32),
            np.asarray(inputs["w_k"], dtype=np.float32),
            np.asarray(inputs["w_v"], dtype=np.float32),
            np.asarray(inputs["w_o"], dtype=np.float32),
            np.asarray(inputs["proj_e"], dtype=np.float32),
            np.asarray(inputs["proj_f"], dtype=np.float32),
        ),
        core_ids=list(range(8)),
    )
    y = np.empty((4, SEQ, FEAT), np.float32)
    for b in range(4):
        y[b] = res.results[2 * b]["y"] + res.results[2 * b + 1]["y"]
    return y


# revision 37
# speedup vs baseline: 1.0340x; 1.0022x over previous
"""Linformer multi-head self-attention on 8 Trainium2 NeuronCores.

Sharding: data-parallel over batch (4) x tensor-parallel over head groups (2).
Core c handles batch c//2, heads (c%2)*8 .. (c%2)*8+8 (channel block of 512).
Each core computes a partial output [4096, 1024]; the host sums the two
partials per batch.

Per-core algorithm (Linformer algebraic reformulation — K and V are never
materialized; only their low-rank projections are):
  C. qT[j, n]      = wq_slice @ x.T for ALL n (runs FIRST: needs only wq +
                     xT, so the PE starts ~7us after launch while the A-phase
                     inputs stream in behind it)
  A. XtEF[f, e2]   = x.T @ [proj_e | proj_f]            (contract n)
  B. kpT[d, e]     = wk_slice @ XtE   (per head-pair)   (contract f)
     vp[e, d]      = XtF.T @ wv_slice.T                 (contract f)
     vaug          = [1 | 0*63 | vp] per head: E-matmul output row 0 is the
                     softmax denominator, rows 64:128 the unnormalized oT
  D. sT[e, n]      = kpT.T @ qT  per head -> one 2-bank PSUM tile; a single
                     exp(sT/8) activation covers both e-chunks (bounded
                     scores, max-subtraction provably unnecessary here)
  E. ps[0|64:128]  = vaug.T @ expT  (row 0 = denominator, rows 64:128 = oT)
     norm: reciprocal_approx_fast (DVE) -> partition_broadcast (Pool) ->
     tensor_mul (DVE).  No PE broadcast matmul, no full-rate reciprocal.
  F. y[n, g]      += outT.T @ wo_slice.T                (contract j)

D/E/F are software-pipelined over 64 (block, head) slots: slot i emits
mul(i-2), D(i), E(i-1)+recip+bcast, F(i-12) so the tensor engine always has
ready work queued ahead of any instruction that waits on ACT/DVE consumers.
The mul is deferred one slot relative to its recip/bcast so the in-order
DVE queue never parks a ready reciprocal behind a mul that is still waiting
on the Pool broadcast (that head-of-line block was worth ~10us).
PSUM: 2x 2-bank score tiles + 2 attention-out banks + 2 output banks.
Block 0 of C' accumulates in two fc half-passes so the very first matmul
needs only the first half of the wq/xT0 DMAs.
"""

import sys

sys.path.insert(0, "/opt/trn_rl_repo")

import numpy as np
import ml_dtypes

import concourse.bass as bass  # noqa: F401  (AP helpers)
import concourse.mybir as mybir
import concourse.tile as tile
from concourse import bacc
from concourse.bass_utils import run_bass_kernel_spmd

SEQ = 4096
FEAT = 1024
PD = 256          # linformer projection dim
J = 512           # per-core head channels (8 heads x 64)
HD = 64           # head dim
NB = 512          # token block for fused loop
N_BLOCKS = SEQ // NB          # 8
NT_PER_BLOCK = NB // 128      # 4
FC = FEAT // 128              # 8 feature chunks
TAU_INV = 1.0 / 8.0           # 1/sqrt(HD)
F_LAG = 11
HEAD_ORDER = [0, 2, 4, 6, 1, 3, 5, 7]

BF16 = mybir.dt.bfloat16
F32 = mybir.dt.float32
NPBF16 = ml_dtypes.bfloat16


def build_nc():
    nc = bacc.Bacc("TRN2", target_bir_lowering=False, debug=False)

    xT = nc.dram_tensor("xT", [FEAT, SEQ], BF16, kind="ExternalInput")
    xn = nc.dram_tensor("xn", [SEQ, FEAT], BF16, kind="ExternalInput")
    pef = nc.dram_tensor("pef", [SEQ, 2 * PD], BF16, kind="ExternalInput")
    wqT = nc.dram_tensor("wqT", [FEAT, J], BF16, kind="ExternalInput")
    wkT = nc.dram_tensor("wkT", [FEAT, J], BF16, kind="ExternalInput")
    wvT = nc.dram_tensor("wvT", [FEAT, J], BF16, kind="ExternalInput")
    woT = nc.dram_tensor("woT", [J, FEAT], BF16, kind="ExternalInput")
    y = nc.dram_tensor("y", [SEQ, FEAT], F32, kind="ExternalOutput")

    with tile.TileContext(nc) as tc:
        _body(nc, tc, xT, xn, pef, wqT, wkT, wvT, woT, y)
    nc.compile()
    return nc


def _body(nc, tc, xT, xn, pef, wqT, wkT, wvT, woT, y):
    Exp = mybir.ActivationFunctionType.Exp

    with (
        tc.tile_pool(name="consts", bufs=1) as consts,
        tc.tile_pool(name="xn_pool", bufs=4) as xn_pool,
        tc.tile_pool(name="pef_pool", bufs=4) as pef_pool,
        tc.tile_pool(name="xtnb", bufs=5) as xtnb_pool,
        tc.tile_pool(name="expp", bufs=3) as exp_pool,
        tc.tile_pool(name="recp", bufs=2) as rec_pool,
        tc.tile_pool(name="bcp", bufs=3) as bc_pool,
        tc.tile_pool(name="outt", bufs=3) as outt_pool,
        tc.tile_pool(name="yp", bufs=3) as y_pool,
    ):
        # ---- resident constants -------------------------------------------
        wq_sb = consts.tile([128, FC, J], BF16, tag="wq")
        wk_sb = consts.tile([128, FC, J], BF16, tag="wk")
        wv_sb = consts.tile([128, FC, J], BF16, tag="wv")
        wo_sb = consts.tile([128, 4, FEAT], BF16, tag="wo")

        xtef_sb = consts.tile([128, FC, 2 * PD], BF16, tag="xtef")
        kpt_sb = consts.tile([128, 4, PD], BF16, tag="kpt")
        # E-matmul lhsT per (ec, head): col 0 = ones (denominator row),
        # cols 1:64 = zeros, cols 64:128 = vp (attention-out rows 64:128)
        vaug_sb = consts.tile([128, 2, 8, 128], BF16, tag="vaug")
        nc.vector.memset(vaug_sb[:], 0.0)
        nc.vector.memset(vaug_sb[:, :, :, 0:1], 1.0)
        qt_all = consts.tile([128, 4, SEQ], BF16, tag="qt_all")

        xn3 = xn[:].rearrange("(t p) f -> p t f", p=128)    # [128, 32, 1024]
        pef3 = pef[:].rearrange("(t p) e -> p t e", p=128)  # [128, 32, 512]
        y3 = y[:].rearrange("(t p) g -> p t g", p=128)      # [128, 32, 1024]
        xT3 = xT[:].rearrange("(c p) n -> p c n", p=128)    # [128, 8, 4096]

        # C' needs only wq chunk 0 + the first xT slice to start: split both
        # loads so the first matmul isn't gated on the full 2 MB
        wqT3 = wqT[:].rearrange("(c p) j -> p c j", p=128)

        # ---- phase C': qT for ALL tokens (PE warm-up phase) ---------------
        # A-phase inputs (12 MB of xn/pef) and remaining weights stream in
        # behind C''s compute.
        xn_tiles = []
        pef_tiles = []
        xt_tiles = []

        def _load_xt(nb):
            xt_nb = xtnb_pool.tile([128, FC, NB], BF16, tag="xtnb", name=f"xt{nb}")
            nc.sync.dma_start(out=xt_nb[:], in_=xT3[:, :, nb * NB : (nb + 1) * NB])
            xt_tiles.append(xt_nb)

        def _load_xnpef(q):
            xn_t = xn_pool.tile([128, 4, FEAT], BF16, tag="xn", name=f"xn{q}")
            pef_t = pef_pool.tile([128, 4, 2 * PD], BF16, tag="pef", name=f"pef{q}")
            nc.sync.dma_start(out=xn_t[:], in_=xn3[:, q * 4 : (q + 1) * 4, :])
            nc.sync.dma_start(out=pef_t[:], in_=pef3[:, q * 4 : (q + 1) * 4, :])
            xn_tiles.append(xn_t)
            pef_tiles.append(pef_t)

        xt0 = xtnb_pool.tile([128, FC, NB], BF16, tag="xtnb", name="xt0")
        for half in range(2):
            nc.sync.dma_start(
                out=wq_sb[:, 4 * half : 4 * half + 4, :],
                in_=wqT3[:, 4 * half : 4 * half + 4, :],
            )
            nc.sync.dma_start(
                out=xt0[:, 4 * half : 4 * half + 4, :],
                in_=xT3[:, 4 * half : 4 * half + 4, 0:NB],
            )
        xt_tiles.append(xt0)
        for nb in range(1, 4):
            _load_xt(nb)
        with tc.tile_pool(name="psC", bufs=1, space="PSUM") as psC_pool:
            for nb in range(N_BLOCKS):
                xt_nb = xt_tiles[nb]
                # stream later xT blocks and the A/B/F-phase inputs behind
                # this block's compute, in need-order
                if nb < 4:
                    _load_xt(nb + 4)
                if nb == 0:
                    _load_xnpef(0)
                    _load_xnpef(1)
                elif nb == 1:
                    _load_xnpef(2)
                    _load_xnpef(3)
                elif nb == 2:
                    nc.sync.dma_start(
                        out=wk_sb[:], in_=wkT[:].rearrange("(c p) j -> p c j", p=128)
                    )
                    nc.sync.dma_start(
                        out=wv_sb[:], in_=wvT[:].rearrange("(c p) j -> p c j", p=128)
                    )
                elif nb == 3:
                    nc.sync.dma_start(
                        out=wo_sb[:], in_=woT[:].rearrange("(c p) g -> p c g", p=128)
                    )
                qtiles = [
                    psC_pool.tile(
                        [128, NB], F32, tag=f"q{(nb % 2) * 4 + jc}", name=f"q{nb}_{jc}"
                    )
                    for jc in range(4)
                ]
                # block 0: two half-passes over fc so the first matmuls need
                # only the first halves of the wq/xT0 loads
                fc_passes = [range(0, 4), range(4, 8)] if nb == 0 else [range(FC)]
                for fcs in fc_passes:
                    for jc in range(4):
                        for fc in fcs:
                            nc.tensor.matmul(
                                qtiles[jc][:],
                                lhsT=wq_sb[:, fc, jc * 128 : (jc + 1) * 128],
                                rhs=xt_nb[:, fc, :],
                                start=(fc == 0),
                                stop=(fc == FC - 1),
                            )
                for jc in range(4):
                    dst = qt_all[:, jc, nb * NB : (nb + 1) * NB]
                    if jc % 2 == 0:
                        nc.vector.tensor_copy(out=dst, in_=qtiles[jc][:])
                    else:
                        nc.scalar.copy(out=dst, in_=qtiles[jc][:])

        # ---- phase A: XtEF = x.T @ [pe|pf] --------------------------------
        with tc.tile_pool(name="psA", bufs=1, space="PSUM") as psA_pool:
            psA = [
                psA_pool.tile([128, 2 * PD], F32, tag=f"ef{i}", name=f"ef{i}")
                for i in range(FC)
            ]
            for q in range(8):
                if q < 4:
                    xn_t, pef_t = xn_tiles[q], pef_tiles[q]
                else:
                    xn_t = xn_pool.tile([128, 4, FEAT], BF16, tag="xn", name=f"xn{q}")
                    pef_t = pef_pool.tile([128, 4, 2 * PD], BF16, tag="pef", name=f"pef{q}")
                    nc.sync.dma_start(out=xn_t[:], in_=xn3[:, q * 4 : (q + 1) * 4, :])
                    nc.sync.dma_start(out=pef_t[:], in_=pef3[:, q * 4 : (q + 1) * 4, :])
                for t in range(4):
                    nci = q * 4 + t
                    for fc in range(FC):
                        nc.tensor.matmul(
                            psA[fc][:],
                            lhsT=xn_t[:, t, fc * 128 : (fc + 1) * 128],
                            rhs=pef_t[:, t, :],
                            start=(nci == 0),
                            stop=(nci == 31),
                        )
            for fc in range(FC):
                if fc % 2 == 0:
                    nc.vector.tensor_copy(out=xtef_sb[:, fc, :], in_=psA[fc][:])
                else:
                    nc.scalar.copy(out=xtef_sb[:, fc, :], in_=psA[fc][:])

        # ---- phase B: kpT (per head pair) and vaug ------------------------
        with tc.tile_pool(name="psB", bufs=2, space="PSUM") as psB_pool:
            def _emit_kp(p):
                ps = psB_pool.tile([128, PD], F32, tag="kp", name=f"kp{p}")
                for fc in range(FC):
                    nc.tensor.matmul(
                        ps[:],
                        lhsT=wk_sb[:, fc, p * 128 : (p + 1) * 128],
                        rhs=xtef_sb[:, fc, 0:PD],
                        start=(fc == 0),
                        stop=(fc == FC - 1),
                    )
                if p % 2 == 0:
                    nc.vector.tensor_copy(out=kpt_sb[:, p, :], in_=ps[:])
                else:
                    nc.scalar.copy(out=kpt_sb[:, p, :], in_=ps[:])

            def _emit_vp(ec):
                ps = psB_pool.tile([128, J], F32, tag="vp", name=f"vp{ec}")
                for fc in range(FC):
                    nc.tensor.matmul(
                        ps[:],
                        lhsT=xtef_sb[:, fc, PD + ec * 128 : PD + (ec + 1) * 128],
                        rhs=wv_sb[:, fc, :],
                        start=(fc == 0),
                        stop=(fc == FC - 1),
                    )
                for h in range(8):
                    if h % 2 == 0:
                        nc.vector.tensor_copy(
                            out=vaug_sb[:, ec, h, 64:128],
                            in_=ps[:, h * HD : (h + 1) * HD],
                        )
                    else:
                        nc.scalar.copy(
                            out=vaug_sb[:, ec, h, 64:128],
                            in_=ps[:, h * HD : (h + 1) * HD],
                        )

            _emit_kp(0)
            _emit_vp(0)
            _emit_kp(1)
            _emit_vp(1)
            _emit_kp(2)
            _emit_kp(3)

        # ---- fused D/E/F per token block of 512, software-pipelined -------
        N_SLOTS = N_BLOCKS * 8
        with tc.tile_pool(name="psM", bufs=1, space="PSUM") as psM_pool:
            outt_tiles = {}
            ysb_cur = [None]

            def emit_D(i):
                nb, h = i // 8, HEAD_ORDER[i % 8]
                p, off = h // 2, (h % 2) * 64
                ex = exp_pool.tile([128, 2, NB], BF16, tag="exp", name=f"ex{i}")
                ps_s = psM_pool.tile(
                    [128, 2, NB], F32, tag=f"s{i % 2}", name=f"ps_s{i}"
                )
                for ec in range(2):
                    nc.tensor.matmul(
                        ps_s[:, ec, :],
                        lhsT=kpt_sb[off : off + 64, p, ec * 128 : (ec + 1) * 128],
                        rhs=qt_all[off : off + 64, p, nb * NB : (nb + 1) * NB],
                        start=True,
                        stop=True,
                    )
                nc.scalar.activation(
                    out=ex[:], in_=ps_s[:], func=Exp, scale=TAU_INV
                )
                return ex

            ps_o_tiles = {}
            bc_tiles = {}

            def emit_E_rb(i, ex):
                nb, h = i // 8, HEAD_ORDER[i % 8]
                if i % 8 == 0:
                    outt_tiles[nb] = outt_pool.tile(
                        [128, 4, NB], BF16, tag="outt", name=f"outt{nb}"
                    )
                ps_o = psM_pool.tile([128, NB], F32, tag=f"o{i % 2}", name=f"ps_o{i}")
                ps_o_tiles[i] = ps_o
                for ec in range(2):
                    nc.tensor.matmul(
                        ps_o[:],
                        lhsT=vaug_sb[:, ec, h, :],
                        rhs=ex[:, ec, :],
                        start=(ec == 0),
                        stop=(ec == 1),
                    )
                rec = rec_pool.tile([1, NB], F32, tag="rec", name=f"rec{i}")
                bc_sb = bc_pool.tile([HD, NB], F32, tag="bc", name=f"bc{i}")
                bc_tiles[i] = bc_sb
                nc.vector.reciprocal_approx_fast(out=rec[:], in_=ps_o[0:1, :])
                nc.gpsimd.partition_broadcast(bc_sb[:], rec[:])

            def emit_mul(i):
                nb, h = i // 8, HEAD_ORDER[i % 8]
                p, off = h // 2, (h % 2) * 64
                nc.vector.tensor_mul(
                    out=outt_tiles[nb][off : off + 64, p, :],
                    in0=ps_o_tiles.pop(i)[64:128, :],
                    in1=bc_tiles.pop(i),
                )

            def emit_F_group(i):
                nb, g = i // 8, i % 8
                tl, gh = g // 2, g % 2
                outt_nb = outt_tiles[nb]
                if gh == 0:
                    ysb_cur[0] = y_pool.tile([128, FEAT], F32, tag="y", name=f"ysb{i}")
                ysb = ysb_cur[0]
                ps_f = psM_pool.tile([128, NB], F32, tag=f"f{g % 2}", name=f"ps_f{i}")
                for p in range(4):
                    nc.tensor.matmul(
                        ps_f[:],
                        lhsT=outt_nb[:, p, tl * 128 : (tl + 1) * 128],
                        rhs=wo_sb[:, p, gh * NB : (gh + 1) * NB],
                        start=(p == 0),
                        stop=(p == 3),
                    )
                nc.scalar.copy(out=ysb[:, gh * NB : (gh + 1) * NB], in_=ps_f[:])
                if gh == 1:
                    nc.sync.dma_start(
                        out=y3[:, nb * NT_PER_BLOCK + tl, :], in_=ysb[:]
                    )

            ex_prev = None
            for i in range(N_SLOTS + F_LAG):
                if i < N_SLOTS:
                    ex_cur = emit_D(i)
                if i >= 2 and i - 2 < N_SLOTS:
                    emit_mul(i - 2)
                if i >= 1 and i - 1 < N_SLOTS:
                    emit_E_rb(i - 1, ex_prev)
                if i >= F_LAG:
                    emit_F_group(i - F_LAG)
                if i < N_SLOTS:
                    ex_prev = ex_cur


_NC_CACHE = {}


def _get_nc():
    if "nc" not in _NC_CACHE:
        _NC_CACHE["nc"] = build_nc()
    return _NC_CACHE["nc"]


def _in_maps(x, w_q, w_k, w_v, w_o, proj_e, proj_f):
    pef = np.concatenate([proj_e, proj_f], axis=1).astype(NPBF16)
    maps = []
    for c in range(8):
        b, hg = c // 2, c % 2
        xb = np.asarray(x[b], dtype=np.float32)
        sl = slice(hg * J, (hg + 1) * J)
        maps.append(
            {
                "xT": xb.T.astype(NPBF16),
                "xn": xb.astype(NPBF16),
                "pef": pef,
                "wqT": w_q[sl, :].T.astype(NPBF16),
                "wkT": w_k[sl, :].T.astype(NPBF16),
                "wvT": w_v[sl, :].T.astype(NPBF16),
                "woT": w_o[:, sl].T.astype(NPBF16),
            }
        )
    return maps


def kernel(**inputs):
    x = np.asarray(inputs["x"], dtype=np.float32)
    res = run_bass_kernel_spmd(
        _get_nc(),
        _in_maps(
            x,
            np.asarray(inputs["w_q"], dtype=np.float32),
            np.asarray(inputs["w_k"], dtype=np.float32),
            np.asarray(inputs["w_v"], dtype=np.float32),
            np.asarray(inputs["w_o"], dtype=np.float32),
            np.asarray(inputs["proj_e"], dtype=np.float32),
            np.asarray(inputs["proj_f"], dtype=np.float32),
        ),
        core_ids=list(range(8)),
    )
    y = np.empty((4, SEQ, FEAT), np.float32)
    for b in range(4):
        y[b] = res.results[2 * b]["y"] + res.results[2 * b + 1]["y"]
    return y
